# revision 1
# baseline (speedup 1.0000x reference)
"""Trainium2 Bass kernel for batched multi-head attention with RoPE + pos_bias.

Reference computation (per batch b):
    qkv = x @ w_qkv ; q,k,v = split(qkv)
    q *= 64**-0.5 ; q,k = rope(q), rope(k)      (interleaved lucidrains RoPE)
    sim = q @ k^T + pos_bias[h]                  (per head)
    out = softmax(sim) @ v ; out @ w_out

Sharding: pure data-parallel over batch — B=8 batches on 8 NeuronCores, no
collectives. Weights / pos_bias / RoPE tables replicated per core.

Per-core design (matmuls in float32r = full PE rate with ~1e-4 rounding):
  - x is pre-transposed on host to xT [512, 1024] so q^T/k^T [64, N] come
    straight out of the QKV matmuls (feature dim on partitions).
  - w_q/w_k columns are de-interleaved per head (evens then odds) so the
    RoPE rotate-half is a 32-partition block swap, done with one PE
    permutation matmul per tile; cos/sin tables (host-precomputed, signed,
    q-tables pre-scaled by 64**-0.5) finish RoPE with [128,1024]-wide
    tensor ops: cos-mul and final add on GpSimd, sin-mul on DVE
    (WIDE_ROPE + WIDE_ADD_POOL); V projection interleaved per-pt so
    attention head pairs unblock as early as possible.
  - Attention runs per HEAD PAIR: the two heads of a pair live in the
    lower/upper 64 partitions of one q^T/k^T chunk, so their K=64 sim
    matmuls lower to PE row-tiles T0/T8 (tile_position auto-inferred) and
    execute concurrently on the split systolic array.
  - S^T tiles [j=128, i=512] (EXP512 mode: 4 single-bank PSUM slots for a
    fine-grained PSUM recycle pipeline). pos_bias is host-transposed per
    head, bf16; added on PE (bf16 identity matmul accumulated into the S
    PSUM bank) for ~56% of tiles and on DVE (tensor_tensor from PSUM,
    which also evicts) for the rest — balances the two engines.
  - exp on ACT writes P^T in bf16; PV uses P^T directly as lhsT with a
    bf16 V that carries a ones column, so out^T_h [65, i] row 64 is the
    softmax denominator for free. Normalization multiplies by a
    PE-broadcast reciprocal row (ones-matmul partition broadcast).
  - attn^T [512, 1024] is exactly the lhsT the output projection needs —
    no transposes anywhere in the kernel.
  - PSUM budget: 4 single-bank S slots + 4 PV banks = 8 banks, with QKV /
    rot / broadcast / final psums time-sharing the same tags.

Measured on TRN2 (8 cores): max rel err 2.3e-3 vs the f32 jax reference.
"""

import sys

for _p in ("/opt/trn_rl_repo",):
    if _p not in sys.path:
        sys.path.insert(0, _p)

import numpy as np
import ml_dtypes

import concourse.bass as bass
import concourse.bacc as bacc
import concourse.tile as tile
from concourse import mybir
from concourse.bass_utils import run_bass_kernel_spmd

F32 = mybir.dt.float32
F32R = mybir.dt.float32r
BF16 = mybir.dt.bfloat16

B, N, DIM = 8, 1024, 512
HEADS, DH = 8, 64
NC_CORES = 8
ROPE_BASE = 10000.0

# ---- tuning knobs ----
# Of the 64 (h, jc) bias units, the fraction handled on PE (bf16 identity
# matmul accumulate) instead of DVE (tensor_tensor add). Spread round-robin.
BIAS_PE_FRAC = 0.5625
PT_BUFS = 6
BIAS_BUFS = 4
WORK_BUFS = 4
SMALL_BUFS = 4
CONST_HIGH_PRIO = False
S_BANKS = 4
POOL_MODE = "stack"
WIDE_ROPE = True
QKV_DUAL = False
NORM_ISL_MAJOR = False
WIDE_ADD_POOL = True  # wide-rope final add on GpSimd
BIAS_DMA_SPLIT = False
ROT_TAG = "pv"
BIAS_PAT = "rr"  # rr | pe_early | pe_late (route pattern within each pair)
BIAS_DMA = "sp_pool"
# ablation flags (debugging only - wrong numerics when enabled)
ABL_NO_ROPE = False
ABL_NO_BIAS = False
ABL_NO_FINAL = False
ABL_NO_QKV = False
ABL_NO_ATTN = False
# structure knobs
EVICT_PV = False         # evict PV psum to SBUF before normalization
ATTNT_TT_ENGINE = "vector"   # engine for attnT = pv * bcast multiply
PV_EVICT_ENGINE = "vector"
PARTIAL_FINAL = False
PV_BF16 = True          # p_t + vh in bf16 (PV matmul bf16)
PV_SPLIT_ISL = False     # accumulate PV isl=0 in-loop, isl=1 in a tail pass
S_SLOTS = 2             # s psum slots (2 banks each)
PV_SLOTS = 4
V_ORDER = "before"
EXP512 = True  # S tiles [128,512] (4 psum slots, exp per 512)  # v projection placement: "before"/"after" each qk group, or all at "end"
# engine for PSUM->SBUF f32r eviction of raw q/k (rot matmul input)
ROT_COPY_ENGINE = "scalar"
# engine for the broadcast-reciprocal PSUM->SBUF copy
BC_COPY_ENGINE = "scalar"
# engine for final out PSUM->SBUF copy
O_COPY_ENGINE = "scalar"
# rope combine ops on gpsimd to offload DVE
ROPE_ON_POOL = True


def _copy_engine(nc, name):
    if name == "scalar" or name == "alt":
        return nc.scalar.copy
    if name == "vector":
        return nc.vector.tensor_copy
    raise ValueError(name)


def _build_nc(reps=1):
    nc = bacc.Bacc("TRN2", num_devices=NC_CORES, debug=False)

    xT = nc.declare_dram_parameter("xT", [DIM, N], F32, isOutput=False)
    wq = nc.declare_dram_parameter("wq", [DIM, DIM], F32, isOutput=False)
    wk = nc.declare_dram_parameter("wk", [DIM, DIM], F32, isOutput=False)
    wv = nc.declare_dram_parameter("wv", [DIM, DIM], F32, isOutput=False)
    wo = nc.declare_dram_parameter("wo", [DIM, DIM], F32, isOutput=False)
    posT = nc.declare_dram_parameter("posT", [HEADS, N, N], BF16, isOutput=False)
    cq = nc.declare_dram_parameter("cq", [128, N], F32, isOutput=False)
    sq = nc.declare_dram_parameter("sq", [128, N], F32, isOutput=False)
    ck = nc.declare_dram_parameter("ck", [128, N], F32, isOutput=False)
    sk = nc.declare_dram_parameter("sk", [128, N], F32, isOutput=False)
    psw = nc.declare_dram_parameter("psw", [128, 128], F32, isOutput=False)
    wq2 = nc.declare_dram_parameter("wq2", [DIM, DIM], F32, isOutput=False)
    wk2 = nc.declare_dram_parameter("wk2", [DIM, DIM], F32, isOutput=False)
    identb = nc.declare_dram_parameter("identb", [128, 128], BF16, isOutput=False)
    out = nc.declare_dram_parameter("out", [N, DIM], F32, isOutput=True)

    n_bias_pe = int(round(64 * BIAS_PE_FRAC))

    with tile.TileContext(nc, pool_alloc_mode=POOL_MODE) as tc:
        with (
            tc.tile_pool(name="const", bufs=1) as cpool,
            tc.tile_pool(name="persist", bufs=1) as ppool,
            tc.tile_pool(name="work", bufs=WORK_BUFS) as wpool,
            tc.tile_pool(name="ptpool", bufs=PT_BUFS) as ptpool,
            tc.tile_pool(name="bias", bufs=BIAS_BUFS) as bpool,
            tc.tile_pool(name="small", bufs=SMALL_BUFS) as small,
        ):
            # ---- constants / weights into SBUF ----
            from contextlib import nullcontext
            _hp = tc.high_priority() if CONST_HIGH_PRIO else nullcontext()
            _hp.__enter__()
            qkv_dt = BF16 if QKV_DUAL else F32R
            xT_sb = cpool.tile([128, 4, N], qkv_dt)
            if QKV_DUAL:
                # gpsimd DMA casts f32 -> bf16 on the fly
                xTv = xT[:, :].rearrange("(o p) n -> p o n", p=128)
                for kc in range(4):
                    nc.gpsimd.dma_start(xT_sb[:, kc], xTv[:, kc])
            else:
                xTv = xT[:, :].bitcast(F32R).rearrange("(o p) n -> p o n", p=128)
                for kc in range(4):
                    nc.sync.dma_start(xT_sb[:, kc], xTv[:, kc])
            w_sbs = {}
            _wq = [nc.scalar, nc.sync, nc.gpsimd, nc.scalar]
            wlist = [("wq", wq), ("wk", wk), ("wv", wv), ("wo", wo)]
            if QKV_DUAL:
                wlist += [("wq2", wq2), ("wk2", wk2)]
            for wi, (name, w) in enumerate(wlist):
                dt_w = F32R if name == "wo" else qkv_dt
                t = cpool.tile([128, 4, DIM], dt_w, name=f"w_{name}", tag=f"w_{name}")
                if dt_w == BF16:
                    wv_view = w[:, :].rearrange("(o p) f -> p o f", p=128)
                    for kc in range(4):
                        nc.gpsimd.dma_start(t[:, kc], wv_view[:, kc])
                else:
                    wv_view = w[:, :].bitcast(F32R).rearrange("(o p) f -> p o f", p=128)
                    for kc in range(4):
                        _wq[(wi + kc) % 4].dma_start(t[:, kc], wv_view[:, kc])
                w_sbs[name] = t
            tabs = {}
            for ti, (name, tab) in enumerate(
                (("cq", cq), ("sq", sq), ("ck", ck), ("sk", sk))
            ):
                t = cpool.tile([128, N], F32, name=f"tab_{name}", tag=f"tab_{name}")
                _wq[ti % 4].dma_start(t[:], tab[:, :])
                tabs[name] = t
            psw_sb = cpool.tile([128, 128], F32R)
            nc.sync.dma_start(psw_sb[:], psw[:, :].bitcast(F32R))
            idb_sb = cpool.tile([128, 128], BF16)
            nc.sync.dma_start(idb_sb[:], identb[:, :])
            ones_sb = cpool.tile([1, 64], F32R)
            nc.vector.memset(ones_sb[:].bitcast(F32), 1.0)
            _hp.__exit__(None, None, None)

            # ---- persistent intermediates ----
            qT = ppool.tile([128, 4, N], F32R)  # roped q^T (feature, n)
            kT = ppool.tile([128, 4, N], F32R)  # roped k^T
            vdt = BF16 if PV_BF16 else F32R
            vh = ppool.tile([128, 8, HEADS, DH + 1], vdt)  # (n%128, n//128, h, d|1)
            attnT = ppool.tile([128, 4, N], F32R)  # attn^T (feature, n)

            ones_col = vh[:, :, :, DH : DH + 1]
            if vdt == F32R:
                ones_col = ones_col.bitcast(F32)
            nc.vector.memset(ones_col, 1.0)

            with tc.tile_pool(name="psum", bufs=2, space="PSUM") as pspool:
                for _rep in range(reps):
                    _emit_body(
                        nc, tc, wpool, ptpool, bpool, small, pspool, xT_sb,
                        w_sbs, tabs, psw_sb, idb_sb, ones_sb, qT, kT, vh,
                        attnT, posT, out, n_bias_pe,
                    )

    return nc


def _bias_on_pe(unit, n_bias_pe):
    if BIAS_PAT == "rr":
        return (unit * n_bias_pe // 64) != ((unit + 1) * n_bias_pe // 64)
    u16 = unit % 16
    n_pair = (n_bias_pe * 16 + 32) // 64  # PE units per pair, rounded
    if BIAS_PAT == "pe_early":
        return u16 < n_pair
    return u16 >= 16 - n_pair


def _s_bufs():
    return S_BANKS if EXP512 else S_SLOTS


def _emit_body(nc, tc, wpool, ptpool, bpool, small, pspool, xT_sb, w_sbs, tabs,
               psw_sb, idb_sb, ones_sb, qT, kT, vh, attnT, posT, out, n_bias_pe):
    rot_copy = _copy_engine(nc, ROT_COPY_ENGINE)
    bc_copy = _copy_engine(nc, BC_COPY_ENGINE)
    o_copy = _copy_engine(nc, O_COPY_ENGINE)

    # ---- QKV projections + RoPE ----
    if True:
        def emit_v(jc):
            ps = pspool.tile([128, 512], F32, tag="s", bufs=_s_bufs(), name="ps_v")
            for kc in range(4):
                nc.tensor.matmul(
                    ps[:],
                    xT_sb[:, kc, jc * 128 : jc * 128 + 128],
                    w_sbs["wv"][:, kc, :],
                    start=(kc == 0),
                    stop=(kc == 3),
                )
            nc.vector.tensor_copy(
                vh[:, jc, :, 0:DH],
                ps[:].rearrange("p (h d) -> p h d", h=HEADS),
            )

        for pt in range(4):
            if V_ORDER == "before":
                emit_v(2 * pt)
                emit_v(2 * pt + 1)
            for tgt, wname, cname, sname in (
                (qT, "wq", "cq", "sq"),
                (kT, "wk", "ck", "sk"),
            ):
                w_sb = w_sbs[wname]
                ct, st = tabs[cname], tabs[sname]
                _rc = rot_copy
                if ROT_COPY_ENGINE == "alt":
                    _rc = nc.scalar.copy if (pt % 2 == 0) else nc.vector.tensor_copy
                if QKV_DUAL:
                    w2_sb = w_sbs[wname + "2"]
                    t1 = wpool.tile([128, 1024], F32, tag="rope_t1", bufs=2)
                    t2 = wpool.tile([128, 1024], F32, tag="rope_t2", bufs=2)
                    for isl in range(2):
                        nsl = slice(isl * 512, isl * 512 + 512)
                        ps = pspool.tile(
                            [128, 512], F32, tag="s", bufs=_s_bufs(), name="ps_qkv"
                        )
                        rps = pspool.tile(
                            [128, 512], F32, tag=ROT_TAG,
                            bufs=PV_SLOTS if ROT_TAG == "pv" else _s_bufs(),
                            name="ps_rot",
                        )
                        for kc in range(4):
                            nc.tensor.matmul(
                                ps[:],
                                w_sb[:, kc, pt * 128 : pt * 128 + 128],
                                xT_sb[:, kc, nsl],
                                start=(kc == 0),
                                stop=(kc == 3),
                            )
                        for kc in range(4):
                            nc.tensor.matmul(
                                rps[:],
                                w2_sb[:, kc, pt * 128 : pt * 128 + 128],
                                xT_sb[:, kc, nsl],
                                start=(kc == 0),
                                stop=(kc == 3),
                            )
                        nc.vector.tensor_tensor(
                            t1[:, nsl], ps[:], ct[:, nsl], mybir.AluOpType.mult
                        )
                        nc.vector.tensor_tensor(
                            t2[:, nsl], rps[:], st[:, nsl], mybir.AluOpType.mult
                        )
                    nc.gpsimd.tensor_tensor(
                        tgt[:, pt, :], t1[:], t2[:], mybir.AluOpType.add
                    )
                    continue
                if WIDE_ROPE:
                    # both i-slices together: fewer, wider elementwise ops
                    pss, rpss = [], []
                    raw = wpool.tile([128, 1024], F32R, tag="qk_raw", bufs=2)
                    for isl in range(2):
                        nsl = slice(isl * 512, isl * 512 + 512)
                        ps = pspool.tile(
                            [128, 512], F32, tag="s", bufs=_s_bufs(), name="ps_qkv"
                        )
                        for kc in range(4):
                            nc.tensor.matmul(
                                ps[:],
                                w_sb[:, kc, pt * 128 : pt * 128 + 128],
                                xT_sb[:, kc, nsl],
                                start=(kc == 0),
                                stop=(kc == 3),
                            )
                        _rc(raw[:, nsl], ps[:])
                        pss.append(ps)
                    t2 = wpool.tile([128, 1024], F32, tag="rope_t2", bufs=2)
                    for isl in range(2):
                        nsl = slice(isl * 512, isl * 512 + 512)
                        rps = pspool.tile(
                            [128, 512], F32, tag=ROT_TAG,
                            bufs=PV_SLOTS if ROT_TAG == "pv" else _s_bufs(),
                            name="ps_rot",
                        )
                        nc.tensor.matmul(
                            rps[:], psw_sb[:], raw[:, nsl], start=True, stop=True
                        )
                        nc.vector.tensor_tensor(
                            t2[:, nsl], rps[:], st[:, nsl], mybir.AluOpType.mult
                        )
                    t1 = wpool.tile([128, 1024], F32, tag="rope_t1", bufs=2)
                    nc.gpsimd.tensor_tensor(
                        t1[:], raw[:], ct[:, :], mybir.AluOpType.mult
                    )
                    add_e = nc.gpsimd if WIDE_ADD_POOL else nc.vector
                    add_e.tensor_tensor(
                        tgt[:, pt, :], t1[:], t2[:], mybir.AluOpType.add
                    )
                    continue
                for isl in range(2):
                    nsl = slice(isl * 512, isl * 512 + 512)
                    ps = pspool.tile([128, 512], F32, tag="s", bufs=_s_bufs(), name="ps_qkv")
                    for kc in range(4):
                        nc.tensor.matmul(
                            ps[:],
                            w_sb[:, kc, pt * 128 : pt * 128 + 128],
                            xT_sb[:, kc, nsl],
                            start=(kc == 0),
                            stop=(kc == 3),
                        )
                    if ABL_NO_ROPE:
                        rot_copy(tgt[:, pt, nsl], ps[:])
                        continue
                    raw = wpool.tile([128, 512], F32R, tag="qk_raw", bufs=2 if WIDE_ROPE else None)
                    rot_copy(raw[:], ps[:])
                    rps = pspool.tile([128, 512], F32, tag="pv", bufs=PV_SLOTS, name="ps_rot")
                    nc.tensor.matmul(
                        rps[:], psw_sb[:], raw[:], start=True, stop=True
                    )
                    t1 = wpool.tile([128, 512], F32, tag="rope_t1")
                    if ROPE_ON_POOL:
                        # gpsimd cannot read PSUM: feed it raw (SBUF)
                        nc.gpsimd.tensor_tensor(
                            t1[:], raw[:], ct[:, nsl], mybir.AluOpType.mult
                        )
                    else:
                        nc.vector.tensor_tensor(
                            t1[:], ps[:], ct[:, nsl], mybir.AluOpType.mult
                        )
                    t2 = wpool.tile([128, 512], F32, tag="rope_t2")
                    nc.vector.tensor_tensor(
                        t2[:], rps[:], st[:, nsl], mybir.AluOpType.mult
                    )
                    nc.vector.tensor_tensor(
                        tgt[:, pt, nsl], t1[:], t2[:], mybir.AluOpType.add
                    )
            if V_ORDER == "after":
                emit_v(2 * pt)
                emit_v(2 * pt + 1)
        if V_ORDER == "end":
            for jc in range(8):
                emit_v(jc)

    # ---- attention, head pairs interleaved (64-row PE tiles T0/T8) ----
    for pg in range(0 if not ABL_NO_ATTN else 4, 4):
        heads = (2 * pg, 2 * pg + 1)
        pt = pg
        rows = (slice(0, 64), slice(64, 128))
        _pvbufs = PV_SLOTS if PV_SPLIT_ISL else PV_SLOTS
        _pvisl = (0,) if PV_SPLIT_ISL else (0, 1)
        pvs = {
            (hi, isl): pspool.tile(
                [DH + 1, 512], F32, tag="pv", bufs=_pvbufs,
                name=f"pv_{pg}_{hi}_{isl}"
            )
            for hi in range(2)
            for isl in _pvisl
        }
        pts = []
        for jc in range(8):
            jsl = slice(jc * 128, jc * 128 + 128)
            s_ps = {}
            bts = {}
            for hi, h in enumerate(heads):
                if EXP512:
                    s_ps[hi] = [
                        pspool.tile(
                            [128, 512], F32, tag="s", bufs=_s_bufs(),
                            name=f"s_ps_{pg}_{hi}_{i}",
                        )
                        for i in range(2)
                    ]
                else:
                    s_ps[hi] = pspool.tile(
                        [128, 1024], F32, tag="s", bufs=_s_bufs(),
                        name=f"s_ps_{pg}_{hi}"
                    )
                bt = bpool.tile([128, 1024], BF16, tag="bias_b")
                if ABL_NO_BIAS:
                    bts[hi] = bt
                    continue
                if BIAS_DMA == "sp_pool":
                    dma_eng = nc.sync if ((jc + hi) % 2 == 0) else nc.gpsimd
                elif BIAS_DMA == "sp_act":
                    dma_eng = nc.sync if ((jc + hi) % 2 == 0) else nc.scalar
                elif BIAS_DMA == "sp":
                    dma_eng = nc.sync
                elif BIAS_DMA == "sp3_pool1":
                    dma_eng = nc.gpsimd if ((jc * 2 + hi) % 4 == 3) else nc.sync
                else:
                    dma_eng = (nc.sync, nc.gpsimd, nc.scalar)[(jc + hi) % 3]
                if BIAS_DMA_SPLIT:
                    dma_eng.dma_start(bt[:, 0:512], posT[h, jsl, 0:512])
                    dma_eng.dma_start(bt[:, 512:1024], posT[h, jsl, 512:1024])
                else:
                    dma_eng.dma_start(bt[:], posT[h, jsl, :])
                bts[hi] = bt
            # paired sim matmuls: T0/T8 row-tiles run concurrently
            for isl in range(2):
                nsl = slice(isl * 512, isl * 512 + 512)
                for hi in range(2):
                    unit = (pg * 8 + jc) * 2 + hi
                    b_pe = _bias_on_pe(unit, n_bias_pe)
                    if ABL_NO_BIAS:
                        b_pe = False
                    tgt_ap = s_ps[hi][isl][:] if EXP512 else s_ps[hi][:, nsl]
                    nc.tensor.matmul(
                        tgt_ap,
                        kT[rows[hi], pt, jsl],
                        qT[rows[hi], pt, nsl],
                        start=True,
                        stop=not b_pe,
                    )
            # bias accumulate (PE bf16 identity or DVE TT) + exp + PV per head
            for hi, h in enumerate(heads):
                unit = (pg * 8 + jc) * 2 + hi
                bias_on_pe = _bias_on_pe(unit, n_bias_pe)
                if ABL_NO_BIAS:
                    bias_on_pe = True
                p_t = ptpool.tile([128, 1024], BF16 if PV_BF16 else F32R, tag="p_t")
                if bias_on_pe:
                    for isl in range(2):
                        if ABL_NO_BIAS:
                            break
                        nsl = slice(isl * 512, isl * 512 + 512)
                        nc.tensor.matmul(
                            s_ps[hi][isl][:] if EXP512 else s_ps[hi][:, nsl],
                            idb_sb[:],
                            bts[hi][:, nsl],
                            start=False,
                            stop=True,
                        )
                    if EXP512:
                        for isl in range(2):
                            nsl = slice(isl * 512, isl * 512 + 512)
                            nc.scalar.activation(
                                p_t[:, nsl], s_ps[hi][isl][:],
                                mybir.ActivationFunctionType.Exp,
                            )
                    else:
                        nc.scalar.activation(
                            p_t[:], s_ps[hi][:], mybir.ActivationFunctionType.Exp
                        )
                else:
                    s_sb = wpool.tile([128, 1024], F32, tag="s_sb")
                    if EXP512:
                        for isl in range(2):
                            nsl = slice(isl * 512, isl * 512 + 512)
                            nc.vector.tensor_tensor(
                                s_sb[:, nsl], s_ps[hi][isl][:], bts[hi][:, nsl],
                                mybir.AluOpType.add,
                            )
                    else:
                        nc.vector.tensor_tensor(
                            s_sb[:], s_ps[hi][:], bts[hi][:], mybir.AluOpType.add
                        )
                    # single wide exp from SBUF: one ACT instruction, and the
                    # s PSUM banks were already freed by the DVE adds
                    nc.scalar.activation(
                        p_t[:], s_sb[:], mybir.ActivationFunctionType.Exp
                    )
                if PV_SPLIT_ISL:
                    pts.append(p_t)
                    nc.tensor.matmul(
                        pvs[(hi, 0)][:],
                        vh[:, jc, h, :],
                        p_t[:, 0:512],
                        start=(jc == 0),
                        stop=(jc == 7),
                    )
                else:
                    for isl in range(2):
                        nsl = slice(isl * 512, isl * 512 + 512)
                        nc.tensor.matmul(
                            pvs[(hi, isl)][:],
                            vh[:, jc, h, :],
                            p_t[:, nsl],
                            start=(jc == 0),
                            stop=(jc == 7),
                        )
        if PV_SPLIT_ISL:
            for hi, h in enumerate(heads):
                pvs[(hi, 1)] = pspool.tile(
                    [DH + 1, 512], F32, tag="pv", bufs=PV_SLOTS,
                    name=f"pv1_{pg}_{hi}"
                )
                for jc in range(8):
                    nc.tensor.matmul(
                        pvs[(hi, 1)][:],
                        vh[:, jc, h, :],
                        pts[jc * 2 + hi][:, 512:1024],
                        start=(jc == 0),
                        stop=(jc == 7),
                    )
        _norm_order = (
            [(hi, isl) for isl in range(2) for hi in range(2)]
            if NORM_ISL_MAJOR
            else [(hi, isl) for hi in range(2) for isl in range(2)]
        )
        for hi, isl in _norm_order:
            h = heads[hi]
            if True:
                nsl = slice(isl * 512, isl * 512 + 512)
                # evict PV psum early to free the slot for the next pair
                if EVICT_PV:
                    pv_sb = wpool.tile([DH + 1, 512], F32, tag="pv_sb", bufs=4)
                    ev_eng = (
                        nc.vector.tensor_copy
                        if PV_EVICT_ENGINE == "vector"
                        else nc.scalar.copy
                    )
                    ev_eng(pv_sb[:], pvs[(hi, isl)][:])
                else:
                    pv_sb = pvs[(hi, isl)]
                with nc.allow_low_precision(reason="softmax denom recip in f32r"):
                    rec_r = small.tile([1, 512], F32R, tag="rec_r")
                    nc.vector.reciprocal(rec_r[:], pv_sb[DH : DH + 1, :])
                bc_ps = pspool.tile([64, 512], F32, tag="s", bufs=_s_bufs(), name="bc_ps")
                nc.tensor.matmul(
                    bc_ps[:], ones_sb[:], rec_r[:], start=True, stop=True
                )
                bc_sb = wpool.tile([64, 512], F32, tag="bc_sb")
                bc_copy(bc_sb[:], bc_ps[:])
                tt_eng = nc.vector if ATTNT_TT_ENGINE == "vector" else nc.gpsimd
                if ATTNT_TT_ENGINE == "pool" and not EVICT_PV:
                    tt_eng = nc.vector  # gpsimd cannot read PSUM
                tt_eng.tensor_tensor(
                    attnT[rows[hi], pt, nsl],
                    pv_sb[0:DH, :],
                    bc_sb[:],
                    mybir.AluOpType.mult,
                )

        if PARTIAL_FINAL:
            for nt in range(8):
                f_ps = pspool.tile([128, 512], F32, tag="s", bufs=_s_bufs(), name="f_ps")
                nc.tensor.matmul(
                    f_ps[:],
                    attnT[:, pt, nt * 128 : nt * 128 + 128],
                    w_sbs["wo"][:, pt, :],
                    start=True,
                    stop=True,
                )
                o_sb = wpool.tile([128, 512], F32, tag="o_sb")
                o_copy(o_sb[:], f_ps[:])
                nc.gpsimd.dma_start(
                    out[nt * 128 : nt * 128 + 128, :],
                    o_sb[:],
                    accum_op=mybir.AluOpType.add,
                )

    # ---- output projection ----
    if not ABL_NO_FINAL and not PARTIAL_FINAL:
        for nt in range(8):
            f_ps = pspool.tile([128, 512], F32, tag="s", bufs=_s_bufs(), name="f_ps")
            for kc in range(4):
                nc.tensor.matmul(
                    f_ps[:],
                    attnT[:, kc, nt * 128 : nt * 128 + 128],
                    w_sbs["wo"][:, kc, :],
                    start=(kc == 0),
                    stop=(kc == 3),
                )
            o_sb = wpool.tile([128, 512], F32, tag="o_sb")
            o_copy(o_sb[:], f_ps[:])
            nc.sync.dma_start(out[nt * 128 : nt * 128 + 128, :], o_sb[:])


def _host_prep(x, pos_bias, w_qkv, w_out):
    """Host-side data layout: shard, transpose, tables. Returns in_maps."""
    x = np.asarray(x, dtype=np.float32)
    pos_bias = np.asarray(pos_bias, dtype=np.float32)
    w_qkv = np.asarray(w_qkv, dtype=np.float32)
    w_out = np.asarray(w_out, dtype=np.float32)

    wq_, wk_, wv_ = np.split(w_qkv, 3, axis=-1)
    # de-interleave RoPE pairs per head: evens then odds
    perm = np.empty(DIM, dtype=np.int64)
    for h in range(HEADS):
        base = h * DH
        perm[base : base + 32] = base + 2 * np.arange(32)
        perm[base + 32 : base + 64] = base + 2 * np.arange(32) + 1
    wq_p = np.ascontiguousarray(wq_[:, perm])
    wk_p = np.ascontiguousarray(wk_[:, perm])
    swap = np.empty(DIM, dtype=np.int64)
    for h in range(HEADS):
        base = h * DH
        swap[base : base + 32] = base + 32 + np.arange(32)
        swap[base + 32 : base + 64] = base + np.arange(32)
    wq2_p = np.ascontiguousarray(wq_p[:, swap])
    wk2_p = np.ascontiguousarray(wk_p[:, swap])
    wv_c = np.ascontiguousarray(wv_)
    wo_c = np.ascontiguousarray(w_out)

    # RoPE tables in de-interleaved row layout, tiled to 128 partitions
    inv = 1.0 / ROPE_BASE ** (np.arange(0, DH, 2, dtype=np.float64) / DH)  # [32]
    ang = np.arange(N, dtype=np.float64)[None, :] * inv[:, None]  # [32, N]
    cos64 = np.concatenate([np.cos(ang), np.cos(ang)], axis=0)  # [64, N]
    sin64 = np.concatenate([-np.sin(ang), np.sin(ang)], axis=0)  # signed
    cos128 = np.tile(cos64, (2, 1)).astype(np.float32)
    sin128 = np.tile(sin64, (2, 1)).astype(np.float32)
    scale = DH**-0.5
    cq_t = np.ascontiguousarray(cos128 * scale)
    sq_t = np.ascontiguousarray(sin128 * scale)
    ck_t = cos128
    sk_t = sin128

    # rotate-half permutation (pure swap of 32-blocks, 2 head-blocks of 64)
    psw_t = np.zeros((128, 128), dtype=np.float32)
    for b0 in (0, 64):
        for i in range(32):
            psw_t[b0 + 32 + i, b0 + i] = 1.0
            psw_t[b0 + i, b0 + 32 + i] = 1.0
    identb_t = np.eye(128, dtype=np.float32).astype(ml_dtypes.bfloat16)

    posT = np.ascontiguousarray(pos_bias.transpose(0, 2, 1)).astype(
        ml_dtypes.bfloat16
    )

    in_maps = []
    for b in range(B):
        in_maps.append(
            {
                "xT": np.ascontiguousarray(x[b].T),
                "wq": wq_p,
                "wk": wk_p,
                "wv": wv_c,
                "wo": wo_c,
                "posT": posT,
                "cq": cq_t,
                "sq": sq_t,
                "ck": ck_t,
                "sk": sk_t,
                "psw": psw_t,
                "wq2": wq2_p,
                "wk2": wk2_p,
                "identb": identb_t,
            }
        )
    return in_maps


_NC_CACHE = {}


def _get_nc():
    if "nc" not in _NC_CACHE:
        nc = _build_nc()
        nc.finalize()
        _NC_CACHE["nc"] = nc
    return _NC_CACHE["nc"]


def kernel(x, pos_bias, w_qkv, w_out, _trace=False, _trace_kwargs=None):
    nc = _get_nc()
    in_maps = _host_prep(x, pos_bias, w_qkv, w_out)
    kw = {}
    if _trace:
        kw = {"trace": True, "trace_kwargs": _trace_kwargs or {}}
    try:
        res = run_bass_kernel_spmd(
            nc, in_maps, core_ids=list(range(NC_CORES)), **kw
        )
    except ModuleNotFoundError:
        # NTFF profile hook unavailable in this environment: run untraced
        res = run_bass_kernel_spmd(nc, in_maps, core_ids=list(range(NC_CORES)))
    out = np.stack([res.results[b]["out"] for b in range(B)], axis=0)
    kernel.last_result = res
    return out



# revision 64
# speedup vs baseline: 1.3908x; 1.3908x over previous
"""Trainium2 Bass kernel for batched multi-head attention with RoPE + pos_bias.

Reference computation (per batch b):
    qkv = x @ w_qkv ; q,k,v = split(qkv)
    q *= 64**-0.5 ; q,k = rope(q), rope(k)      (interleaved lucidrains RoPE)
    sim = q @ k^T + pos_bias[h]                  (per head)
    out = softmax(sim) @ v ; out @ w_out

Sharding: pure data-parallel over batch - B=8 batches on 8 NeuronCores, no
collectives. Weights / pos_bias / RoPE tables replicated per core.

Per-core design (all matmuls bf16; pos_bias partly as fp8):
  - x pre-transposed to xT [512, 1024] bf16; w_q/w_k columns de-interleaved
    per head (evens then odds) so RoPE rotate-half is a 32-partition block
    swap done by one PE permutation matmul per [128,1024] chunk.
  - RoPE elementwise: PSUM evict (DVE, bf16), sin-mul on DVE, cos-mul and
    final add on Pool (all-SBUF bf16).
  - sim per head pair: kT/qT rows in lower/upper 64 partitions, K=64 row
    tiles; S^T [j=128, i=1024] per (head, jc) in a 2-bank PSUM tile.
  - pos_bias three routes (tuned split): 'pe' = fp8 identity-matmul
    accumulate into the S PSUM (bias streamed as fp8e4m3, halving that
    DMA); 'pool'/'dve' = multiply host-precomputed exp(bias) (bf16) into
    p after the exp.  (fp8 DoubleRow would halve the PE cost again and
    passes in isolation, but crashes the full-kernel NEFF on hardware -
    see BIAS_PE_MODE.)
  - exp on ACT [128, 1024] PSUM->SBUF bf16 (the ACT engine is ~exp only;
    at ~66us it is the pacing engine, so everything else hides under it).
  - PV flipped: out[i-tile, 65*hi+d] = sum_j P^T[j, i]^T vh[j, d] with P as
    lhsT, vh (ones-column augmented) as moving -> 65-col matmuls instead of
    512-col, i rides the partition dim (PV drops from 65536 to 33280 PE
    cycles). The denominator lands per-partition => softmax normalization
    is a cheap DVE tensor_scalar with scalar AP, no
    reciprocal-broadcast dance.
  - attn [i, hd] -> PE transpose per [128,128] block -> attnT chunks ->
    output projection, evict, DMA out.
  - Software pipelining: QKV chunks for pt+1 and the deferred per-i-tile
    PV/norm/transpose blocks of pg-1 are interleaved into pg's unit
    stream; the tail staggers PV/finish/projection three deep across the
    idle 's' PSUM slots.

Measured: see test.py (CoreSim cost model == harness exec time).
"""

import sys

for _p in ("/opt/trn_rl_repo",):
    if _p not in sys.path:
        sys.path.insert(0, _p)

import numpy as np
import ml_dtypes

import concourse.bass as bass
import concourse.bacc as bacc
import concourse.tile as tile
from concourse import mybir
from concourse.bass_utils import run_bass_kernel_spmd

F32 = mybir.dt.float32
BF16 = mybir.dt.bfloat16
FP8 = mybir.dt.float8e4

B, N, DIM = 8, 1024, 512
HEADS, DH = 8, 64
NC_CORES = 8
ROPE_BASE = 10000.0

# ---- tuning knobs ----
# per-unit bias route: counts out of 64 units (pg,jc,hi); rest -> 'dve'
N_BIAS_PE = 8
N_BIAS_POOL = 28
# engine for PSUM->SBUF evictions
ROPE_EVICT = "vector"   # raw q/k evict
V_EVICT = "vector"      # vh evict
O_EVICT = "vector"      # final out evict
T_EVICT = "vector"      # transpose evict
S_BUFS = 3              # S psum slots ([128,1024] f32, 2 banks each)
PV_BUFS = 2             # pv/trans psum slots (1 bank each)
# after which jc steps of each pg to interleave a background QKV chunk
BG_POINTS = (1, 4)
PT_BUFS = 36
BIAS_BUFS = 8
WORK_BUFS = 4
# DMA queue rotation for bias tiles (only sync/scalar/gpsimd can DMA)
BIAS_QUEUES = ("sync", "sync", "sync", "gpsimd")
EXP_WARMUP = True
BG_TAG = "pv"        # psum tag for background qkv chunks
TAIL_SWAP = True     # tail: pv accum on "pv" tag, tp/f_ps on "s"
TAIL_EVICT = "vector"  # engine for tail attnT/out evicts
TAIL_O_EVICT = "scalar"  # engine for tail out evicts
PG3_PE_BOOST = 4     # extra pe-route bias units in pg3
# "dr" = fp8 DoubleRow (crashes real HW in full kernel), "fp8" = plain fp8
# identity matmul accumulate, "off" = no PE bias route
BIAS_PE_MODE = "fp8"
MID_TP_TAG = "s"     # psum tag for mid-stream transpose
PRO_MUL = "gpsimd"   # engine for prologue rope cos-mul
PRO_EVICT_K = "vector"  # prologue k evict engine


def _evict_engine(nc, name):
    if name == "scalar":
        return nc.scalar.copy
    if name == "vector":
        return nc.vector.tensor_copy
    if name == "gpsimd":
        return nc.gpsimd.tensor_copy
    raise ValueError(name)


def _dma_queue(nc, name):
    return {"sync": nc.sync, "gpsimd": nc.gpsimd, "vector": nc.vector,
            "scalar": nc.scalar}[name]


def _bias_route(unit):
    """unit in [0,64) -> 'pe' | 'pool' | 'dve' (round-robin interleave)."""
    if BIAS_PE_MODE == "off":
        n_rest = 64
        in_pool = (unit * N_BIAS_POOL // n_rest) != (
            (unit + 1) * N_BIAS_POOL // n_rest)
        return "pool" if in_pool else "dve"
    if unit >= 64 - PG3_PE_BOOST:
        return "pe"
    in_pe = (unit * N_BIAS_PE // 64) != ((unit + 1) * N_BIAS_PE // 64)
    if in_pe:
        return "pe"
    rest = unit - (unit * N_BIAS_PE // 64) - (1 if in_pe else 0)
    n_rest = 64 - N_BIAS_PE
    in_pool = (rest * N_BIAS_POOL // n_rest) != ((rest + 1) * N_BIAS_POOL // n_rest)
    return "pool" if in_pool else "dve"


def _build_nc(reps=1):
    nc = bacc.Bacc("TRN2", num_devices=NC_CORES, debug=False)

    xT = nc.declare_dram_parameter("xT", [DIM, N], BF16, isOutput=False)
    wq = nc.declare_dram_parameter("wq", [DIM, DIM], BF16, isOutput=False)
    wk = nc.declare_dram_parameter("wk", [DIM, DIM], BF16, isOutput=False)
    wv = nc.declare_dram_parameter("wv", [DIM, DIM], BF16, isOutput=False)
    wo = nc.declare_dram_parameter("wo", [DIM, DIM], BF16, isOutput=False)
    posDR = nc.declare_dram_parameter("posDR", [HEADS, 8, 64, 2048], FP8,
                                      isOutput=False)
    posT8 = nc.declare_dram_parameter("posT8", [HEADS, N, N], FP8,
                                      isOutput=False)
    posE = nc.declare_dram_parameter("posE", [HEADS, N, N], BF16, isOutput=False)
    cq = nc.declare_dram_parameter("cq", [128, N], BF16, isOutput=False)
    sq = nc.declare_dram_parameter("sq", [128, N], BF16, isOutput=False)
    ck = nc.declare_dram_parameter("ck", [128, N], BF16, isOutput=False)
    sk = nc.declare_dram_parameter("sk", [128, N], BF16, isOutput=False)
    psw = nc.declare_dram_parameter("psw", [128, 128], BF16, isOutput=False)
    idn = nc.declare_dram_parameter("idn", [128, 128], BF16, isOutput=False)
    idr = nc.declare_dram_parameter("idr", [64, 256], FP8, isOutput=False)
    idn8 = nc.declare_dram_parameter("idn8", [128, 128], FP8, isOutput=False)
    out = nc.declare_dram_parameter("out", [N, DIM], F32, isOutput=True)

    with tile.TileContext(nc, pool_alloc_mode="stack") as tc:
        with (
            tc.tile_pool(name="const", bufs=1) as cpool,
            tc.tile_pool(name="persist", bufs=1) as ppool,
            tc.tile_pool(name="work", bufs=WORK_BUFS) as wpool,
            tc.tile_pool(name="ptpool", bufs=PT_BUFS) as ptpool,
            tc.tile_pool(name="bias", bufs=BIAS_BUFS) as bpool,
            tc.tile_pool(name="small", bufs=4) as small,
        ):
            # ---- constants / weights into SBUF ----
            # DMAs issued in consumption order, round-robin over sync/gpsimd
            # (the scalar queue is kept free: the ACT engine is exp-bound).
            xT_sb = cpool.tile([128, 4, N], BF16)
            xTv = xT[:, :].rearrange("(o p) n -> p o n", p=128)
            w_sbs = {}
            wviews = {}
            for name, w in (("wq", wq), ("wk", wk), ("wv", wv), ("wo", wo)):
                w_sbs[name] = cpool.tile(
                    [128, 4, DIM], BF16, name=f"w_{name}", tag=f"w_{name}"
                )
                wviews[name] = w[:, :].rearrange("(o p) f -> p o f", p=128)
            _qs = [nc.sync, nc.gpsimd, nc.scalar]
            qi = 0

            def _ld(dst, src):
                nonlocal qi
                _qs[qi % 3].dma_start(dst, src)
                qi += 1

            # x / wq / wk chunks each on their own queue so the first QKV
            # matmuls can start ~1us in
            for kc in range(4):
                nc.sync.dma_start(xT_sb[:, kc], xTv[:, kc])
                nc.gpsimd.dma_start(w_sbs["wq"][:, kc], wviews["wq"][:, kc])
                nc.scalar.dma_start(w_sbs["wk"][:, kc], wviews["wk"][:, kc])
            psw_sb = cpool.tile([128, 128], BF16)
            _ld(psw_sb[:], psw[:, :])
            tabs = {}
            for name, tab in (("cq", cq), ("sq", sq), ("ck", ck), ("sk", sk)):
                t = cpool.tile([128, N], BF16, name=f"tab_{name}", tag=f"tab_{name}")
                _ld(t[:], tab[:, :])
                tabs[name] = t
            for kc in range(4):
                _ld(w_sbs["wv"][:, kc], wviews["wv"][:, kc])
            idr_sb = cpool.tile([64, 2, 128], FP8)
            _ld(idr_sb[:], idr[:, :].rearrange("p (i n) -> p i n", i=2))
            idn8_sb = cpool.tile([128, 128], FP8)
            _ld(idn8_sb[:], idn8[:, :])
            idn_sb = cpool.tile([128, 128], BF16)
            _ld(idn_sb[:], idn[:, :])
            for kc in range(4):
                _ld(w_sbs["wo"][:, kc], wviews["wo"][:, kc])

            # ---- persistent intermediates ----
            qT = ppool.tile([128, 4, N], BF16)
            kT = ppool.tile([128, 4, N], BF16)
            vh = ppool.tile([128, 8, HEADS, DH + 1], BF16)
            attnT = ppool.tile([128, 4, N], BF16)  # [hd%128, hd//128, i]

            nc.vector.memset(vh[:, :, :, DH : DH + 1], 1.0)
            if EXP_WARMUP:
                warm = small.tile([1, 8], F32, tag="warm")
                nc.vector.memset(warm[:], 0.0)
                nc.scalar.activation(
                    warm[:], warm[:], mybir.ActivationFunctionType.Exp
                )

            with tc.tile_pool(name="psum", bufs=2, space="PSUM") as pspool:
                for _rep in range(reps):
                    _emit_body(
                        nc, tc, wpool, ptpool, bpool, small, pspool, xT_sb,
                        w_sbs, tabs, psw_sb, idn_sb, idr_sb, idn8_sb, qT, kT,
                        vh, attnT, posDR, posT8, posE, out,
                    )

    return nc


def _emit_body(nc, tc, wpool, ptpool, bpool, small, pspool, xT_sb, w_sbs, tabs,
               psw_sb, idn_sb, idr_sb, idn8_sb, qT, kT, vh, attnT, posDR,
               posT8, posE, out):
    rope_evict = _evict_engine(nc, ROPE_EVICT)
    v_evict = _evict_engine(nc, V_EVICT)
    o_evict = _evict_engine(nc, O_EVICT)
    t_evict = _evict_engine(nc, T_EVICT)
    rows = (slice(0, 64), slice(64, 128))

    # ---- QKV projections + RoPE ----
    def emit_v(jc):
        ps = pspool.tile([128, 512], F32, tag="pv", bufs=PV_BUFS, name="ps_v")
        for kc in range(4):
            nc.tensor.matmul(
                ps[:],
                xT_sb[:, kc, jc * 128 : jc * 128 + 128],
                w_sbs["wv"][:, kc, :],
                start=(kc == 0),
                stop=(kc == 3),
            )
        v_evict(
            vh[:, jc, :, 0:DH],
            ps[:].rearrange("p (h d) -> p h d", h=HEADS),
        )

    def qkv_part1(pt, wname, tag="s", bufs=None, evict=None):
        """QKV matmuls for one [128,1024] chunk + PSUM->SBUF bf16 evict."""
        w_sb = w_sbs[wname]
        bufs = bufs or (S_BUFS if tag == "s" else PV_BUFS)
        evict = evict or rope_evict
        raw = wpool.tile([128, 1024], BF16, tag="qk_raw", bufs=2)
        if tag == "s":
            ps = pspool.tile([128, 1024], F32, tag=tag, bufs=bufs,
                             name="ps_qkv")
            halves = [ps[:, 0:512], ps[:, 512:1024]]
        else:
            halves = [
                pspool.tile([128, 512], F32, tag=tag, bufs=bufs,
                            name=f"ps_qkv_{i}")[:]
                for i in range(2)
            ]
        for isl in range(2):
            nsl = slice(isl * 512, isl * 512 + 512)
            for kc in range(4):
                nc.tensor.matmul(
                    halves[isl],
                    w_sb[:, kc, pt * 128 : pt * 128 + 128],
                    xT_sb[:, kc, nsl],
                    start=(kc == 0),
                    stop=(kc == 3),
                )
            if tag == "s" and isl == 1:
                evict(raw[:], ps[:])
            elif tag != "s":
                evict(raw[:, nsl], halves[isl])
        return raw

    def qkv_part2(pt, raw, tgt, cname, sname, comb=None, mul=None, tag="s",
                  bufs=None):
        """rotate-half perm matmul + rope combine into tgt[:, pt, :]."""
        comb = comb or nc.gpsimd
        mul = mul or comb
        bufs = bufs or (S_BUFS if tag == "s" else PV_BUFS)
        ct, st = tabs[cname], tabs[sname]
        t2 = wpool.tile([128, 1024], BF16, tag="rope_t2", bufs=2)
        if tag == "s":
            rps = pspool.tile([128, 1024], F32, tag=tag, bufs=bufs,
                              name="ps_rot")
            for isl in range(2):
                nsl = slice(isl * 512, isl * 512 + 512)
                nc.tensor.matmul(
                    rps[:, nsl], psw_sb[:], raw[:, nsl], start=True, stop=True
                )
            nc.vector.tensor_tensor(t2[:], rps[:], st[:], mybir.AluOpType.mult)
        else:
            for isl in range(2):
                nsl = slice(isl * 512, isl * 512 + 512)
                rp = pspool.tile([128, 512], F32, tag=tag, bufs=bufs,
                                 name=f"ps_rot_{isl}")
                nc.tensor.matmul(
                    rp[:], psw_sb[:], raw[:, nsl], start=True, stop=True
                )
                nc.vector.tensor_tensor(
                    t2[:, nsl], rp[:], st[:, nsl], mybir.AluOpType.mult
                )
        t1 = wpool.tile([128, 1024], BF16, tag="rope_t1", bufs=2)
        mul.tensor_tensor(t1[:], raw[:], ct[:], mybir.AluOpType.mult)
        comb.tensor_tensor(tgt[:, pt, :], t1[:], t2[:], mybir.AluOpType.add)

    def qkv_rope(pt, tgt, wname, cname, sname):
        qkv_part2(pt, qkv_part1(pt, wname), tgt, cname, sname)

    # prologue: q/k for pt 0, all of V; remaining pts stream in as background
    # work interleaved into the attention loop so PE/ACT stay busy.
    # interleave q/k part1 so k's matmuls run while DVE evicts q; rope
    # combines on DVE here (Pool is busy with phase-A DMAs early on)
    pro_mul = nc.vector if PRO_MUL == "vector" else nc.gpsimd
    pro_ev_k = (nc.scalar.copy if PRO_EVICT_K == "scalar"
                else nc.vector.tensor_copy)
    raw_q = qkv_part1(0, "wq")
    raw_k = qkv_part1(0, "wk", evict=pro_ev_k)
    qkv_part2(0, raw_q, qT, "cq", "sq", mul=pro_mul)
    qkv_part2(0, raw_k, kT, "ck", "sk", mul=pro_mul)

    # ---- attention per head pair ----
    pend = {}  # background state
    pv_blocks = []

    for pg in range(4):
        heads = (2 * pg, 2 * pg + 1)
        pt = pg
        p_tiles = {}
        for jc in range(8):
            jsl = slice(jc * 128, jc * 128 + 128)
            for hi, h in enumerate(heads):
                unit = (pg * 8 + jc) * 2 + hi
                route = _bias_route(unit)
                qnm = BIAS_QUEUES[unit % len(BIAS_QUEUES)]
                dq = _dma_queue(nc, qnm)
                if route == "pe" and BIAS_PE_MODE == "dr":
                    bt = bpool.tile([64, 2, 1024], FP8, tag="bias_dr")
                    dq.dma_start(
                        bt[:],
                        posDR[h, jc].rearrange("p (i n) -> p i n", i=2),
                    )
                elif route == "pe":
                    bt = bpool.tile([128, 1024], FP8, tag="bias_dr")
                    dq.dma_start(bt[:], posT8[h, jsl, :])
                else:
                    et = bpool.tile([128, 1024], BF16, tag="bias_e")
                    dq.dma_start(et[:], posE[h, jsl, :])
                s = pspool.tile([128, 1024], F32, tag="s", bufs=S_BUFS,
                                name=f"s_{pg}_{jc}_{hi}")
                for isl in range(2):
                    nsl = slice(isl * 512, isl * 512 + 512)
                    nc.tensor.matmul(
                        s[:, nsl],
                        kT[rows[hi], pt, jsl],
                        qT[rows[hi], pt, nsl],
                        start=True,
                        stop=(route != "pe"),
                    )
                    if route == "pe" and BIAS_PE_MODE == "dr":
                        nc.tensor.matmul(
                            s[:, nsl],
                            idr_sb[:],
                            bt[:, :, nsl],
                            start=False,
                            stop=True,
                            perf_mode=mybir.MatmulPerfMode.DoubleRow,
                        )
                    elif route == "pe":
                        nc.tensor.matmul(
                            s[:, nsl],
                            idn8_sb[:],
                            bt[:, nsl],
                            start=False,
                            stop=True,
                        )
                p_t = ptpool.tile([128, 1024], BF16, tag="p_t")
                nc.scalar.activation(
                    p_t[:], s[:], mybir.ActivationFunctionType.Exp
                )
                if route == "pool":
                    p_m = ptpool.tile([128, 1024], BF16, tag="p_t")
                    nc.gpsimd.tensor_tensor(
                        p_m[:], p_t[:], et[:], mybir.AluOpType.mult
                    )
                    p_t = p_m
                elif route == "dve":
                    p_m = ptpool.tile([128, 1024], BF16, tag="p_t")
                    nc.vector.tensor_tensor(
                        p_m[:], p_t[:], et[:], mybir.AluOpType.mult
                    )
                    p_t = p_m
                p_tiles[(jc, hi)] = p_t
            if pg == 0 and jc >= 1:
                emit_v(jc - 1)
                if jc == 7:
                    emit_v(7)
            if pv_blocks:
                pv_blocks.pop(0)()
            if pg < 3:
                if jc == 2:
                    pend["q"] = qkv_part1(pt + 1, "wq", tag=BG_TAG)
                elif jc == 4:
                    qkv_part2(pt + 1, pend.pop("q"), qT, "cq", "sq", tag=BG_TAG)
                    pend["k"] = qkv_part1(pt + 1, "wk", tag=BG_TAG)
                elif jc == 6:
                    qkv_part2(pt + 1, pend.pop("k"), kT, "ck", "sk", tag=BG_TAG)

        # PV flipped: per i-tile, accumulate [i, 65*hi + d] over jc.
        # Each i-tile chunk is deferred into the next pg's unit stream.
        def pv_mm(it, tag="pv", bufs=PV_BUFS, pg=pg, heads=heads,
                  p_tiles=p_tiles):
            isl = slice(it * 128, it * 128 + 128)
            pv = pspool.tile([128, 2 * (DH + 1)], F32, tag=tag,
                             bufs=bufs, name=f"pv_{pg}_{it}")
            for hi, h in enumerate(heads):
                for jc in range(8):
                    nc.tensor.matmul(
                        pv[:, hi * (DH + 1) : (hi + 1) * (DH + 1)],
                        p_tiles[(jc, hi)][:, isl],
                        vh[:, jc, h, :],
                        start=(jc == 0),
                        stop=(jc == 7),
                    )
            return pv

        def pv_fin(it, pv, tp_tag=None, tp_bufs=None, evict=None, pg=pg,
                   heads=heads):
            tp_tag = tp_tag or MID_TP_TAG
            tp_bufs = tp_bufs or (S_BUFS if tp_tag == "s" else PV_BUFS)
            evict = evict or t_evict
            rec = small.tile([128, 2, 1], F32, tag="rec")
            nc.vector.reciprocal(
                rec[:],
                pv[:].rearrange("p (h d) -> p h d", h=2)[:, :, DH : DH + 1],
            )
            attn_w = wpool.tile([128, 128], BF16, tag="attn_w", bufs=3)
            for hi, h in enumerate(heads):
                nc.vector.tensor_scalar(
                    attn_w[:, hi * DH : (hi + 1) * DH],
                    pv[:, hi * (DH + 1) : hi * (DH + 1) + DH],
                    rec[:, hi, :],
                    None,
                    mybir.AluOpType.mult,
                )
            # transpose head pair's attn chunk into attnT[:, pg, ...]
            tp = pspool.tile([128, 128], BF16, tag=tp_tag, bufs=tp_bufs,
                             name=f"tp_{pg}_{it}")
            nc.tensor.transpose(tp[:], attn_w[:], idn_sb[:])
            evict(attnT[:, pg, it * 128 : it * 128 + 128], tp[:])

        def pv_it(it):
            pv_fin(it, pv_mm(it))

        def proj_it(it, tag="s", bufs=S_BUFS, evict=None):
            evict = evict or o_evict
            f_ps = pspool.tile([128, 512], F32, tag=tag, bufs=bufs,
                               name=f"f_{it}")
            for c in range(4):
                nc.tensor.matmul(
                    f_ps[:],
                    attnT[:, c, it * 128 : it * 128 + 128],
                    w_sbs["wo"][:, c, :],
                    start=(c == 0),
                    stop=(c == 3),
                )
            o_sb = wpool.tile([128, 512], F32, tag="o_sb", bufs=3)
            evict(o_sb[:], f_ps[:])
            nc.sync.dma_start(out[it * 128 : it * 128 + 128, :], o_sb[:])

        if pg == 3:
            # staggered drain on the tail: PV tiles live in the (now idle)
            # 's' slots 3 deep; transposes/projections use the 'pv' slots.
            # Projections lag two i-tiles behind the PV front.
            t_ev = (nc.scalar.copy if TAIL_EVICT == "scalar"
                    else nc.vector.tensor_copy)
            o_ev = (nc.scalar.copy if TAIL_O_EVICT == "scalar"
                    else nc.vector.tensor_copy)
            if TAIL_SWAP:
                acc_tag, acc_bufs, io_tag, io_bufs = "pv", PV_BUFS, "s", S_BUFS
            else:
                acc_tag, acc_bufs, io_tag, io_bufs = "s", S_BUFS, "pv", PV_BUFS
            depth = acc_bufs
            pvs = {}
            for it in range(min(depth, 8)):
                pvs[it] = pv_mm(it, tag=acc_tag, bufs=acc_bufs)
            for it in range(8):
                pv_fin(it, pvs.pop(it), tp_tag=io_tag, tp_bufs=io_bufs,
                       evict=t_ev)
                if it + depth < 8:
                    pvs[it + depth] = pv_mm(it + depth, tag=acc_tag,
                                            bufs=acc_bufs)
                if it - 2 >= 0:
                    proj_it(it - 2, tag=io_tag, bufs=io_bufs, evict=o_ev)
            proj_it(6, tag=io_tag, bufs=io_bufs, evict=o_ev)
            proj_it(7, tag=io_tag, bufs=io_bufs, evict=o_ev)
        else:
            for it in range(8):
                pv_blocks.append(lambda it=it, f=pv_it: f(it))


def _host_prep(x, pos_bias, w_qkv, w_out):
    """Host-side data layout: transpose, de-interleave, tables, fp8 bias."""
    x = np.asarray(x, dtype=np.float32)
    pos_bias = np.asarray(pos_bias, dtype=np.float32)
    w_qkv = np.asarray(w_qkv, dtype=np.float32)
    w_out = np.asarray(w_out, dtype=np.float32)

    wq_, wk_, wv_ = np.split(w_qkv, 3, axis=-1)
    # de-interleave RoPE pairs per head: evens then odds
    perm = np.empty(DIM, dtype=np.int64)
    for h in range(HEADS):
        base = h * DH
        perm[base : base + 32] = base + 2 * np.arange(32)
        perm[base + 32 : base + 64] = base + 2 * np.arange(32) + 1
    wq_p = np.ascontiguousarray(wq_[:, perm]).astype(ml_dtypes.bfloat16)
    wk_p = np.ascontiguousarray(wk_[:, perm]).astype(ml_dtypes.bfloat16)
    wv_c = np.ascontiguousarray(wv_).astype(ml_dtypes.bfloat16)
    wo_c = np.ascontiguousarray(w_out).astype(ml_dtypes.bfloat16)

    # RoPE tables in de-interleaved row layout, tiled to 128 partitions
    inv = 1.0 / ROPE_BASE ** (np.arange(0, DH, 2, dtype=np.float64) / DH)
    ang = np.arange(N, dtype=np.float64)[None, :] * inv[:, None]  # [32, N]
    cos64 = np.concatenate([np.cos(ang), np.cos(ang)], axis=0)  # [64, N]
    sin64 = np.concatenate([-np.sin(ang), np.sin(ang)], axis=0)  # signed
    cos128 = np.tile(cos64, (2, 1)).astype(np.float32)
    sin128 = np.tile(sin64, (2, 1)).astype(np.float32)
    scale = DH**-0.5
    cq_t = (cos128 * scale).astype(ml_dtypes.bfloat16)
    sq_t = (sin128 * scale).astype(ml_dtypes.bfloat16)
    ck_t = cos128.astype(ml_dtypes.bfloat16)
    sk_t = sin128.astype(ml_dtypes.bfloat16)

    # rotate-half permutation (swap 32-blocks within each 64-block)
    psw_t = np.zeros((128, 128), dtype=np.float32)
    for b0 in (0, 64):
        for i in range(32):
            psw_t[b0 + 32 + i, b0 + i] = 1.0
            psw_t[b0 + i, b0 + 32 + i] = 1.0
    psw_t = psw_t.astype(ml_dtypes.bfloat16)

    posT = np.ascontiguousarray(pos_bias.transpose(0, 2, 1))  # [H, j, i]
    # DR packing: posDR[h, jc, k, i2*1024 + n] = posT[h, jc*128 + k + 64*i2, n]
    posDR = (
        posT.reshape(HEADS, 8, 2, 64, N)
        .transpose(0, 1, 3, 2, 4)
        .reshape(HEADS, 8, 64, 2048)
    ).astype(ml_dtypes.float8_e4m3)
    posE = np.exp(posT).astype(ml_dtypes.bfloat16)
    posT8 = posT.astype(ml_dtypes.float8_e4m3)

    idr_t = np.zeros((64, 256), dtype=np.float32)
    for k in range(64):
        idr_t[k, k] = 1.0
        idr_t[k, 128 + 64 + k] = 1.0
    idr_t = idr_t.astype(ml_dtypes.float8_e4m3)
    idn_t = np.eye(128, dtype=np.float32).astype(ml_dtypes.bfloat16)
    idn8_t = np.eye(128, dtype=np.float32).astype(ml_dtypes.float8_e4m3)

    in_maps = []
    for b in range(B):
        in_maps.append(
            {
                "xT": np.ascontiguousarray(x[b].T).astype(ml_dtypes.bfloat16),
                "wq": wq_p,
                "wk": wk_p,
                "wv": wv_c,
                "wo": wo_c,
                "posDR": posDR,
                "posT8": posT8,
                "posE": posE,
                "cq": cq_t,
                "sq": sq_t,
                "ck": ck_t,
                "sk": sk_t,
                "psw": psw_t,
                "idn": idn_t,
                "idn8": idn8_t,
                "idr": idr_t,
            }
        )
    return in_maps


_NC_CACHE = {}


def _get_nc():
    if "nc" not in _NC_CACHE:
        nc = _build_nc()
        nc.finalize()
        _NC_CACHE["nc"] = nc
    return _NC_CACHE["nc"]


def kernel(x, pos_bias, w_qkv, w_out, _trace=False, _trace_kwargs=None):
    nc = _get_nc()
    in_maps = _host_prep(x, pos_bias, w_qkv, w_out)
    kw = {}
    if _trace:
        kw = {"trace": True, "trace_kwargs": _trace_kwargs or {}}
    try:
        res = run_bass_kernel_spmd(
            nc, in_maps, core_ids=list(range(NC_CORES)), **kw
        )
    except ModuleNotFoundError:
        res = run_bass_kernel_spmd(nc, in_maps, core_ids=list(range(NC_CORES)))
    out = np.stack([res.results[b]["out"] for b in range(B)], axis=0)
    kernel.last_result = res
    return out
